# revision 16
# baseline (speedup 1.0000x reference)
"""Trainium2 Bass kernel for nn_BotNetwork (dense MLP + action-mask).

Contract: kernel(**inputs) takes FULL unsharded inputs (B=262144 rows),
shards the batch across 8 NeuronCores (pure data parallelism), runs a
Bass/Tile kernel per core, and gathers the full [B, 9] output.

Device-side layout notes:
 - Rows are processed in groups of 128*G. A group's [128*G, 121] slab is
   DMA'd as one contiguous [128, G*121] SBUF tile: partition p holds rows
   p*G..p*G+G-1 of the group.
 - Matmuls run feature-major: row-major tiles are PE-transposed into
   [121, 128*G] activations; the MLP chain stays feature-major until the
   last layer, which is computed "flipped" (activations stationary) so
   logits land row-major [128, G*9] in PSUM.
 - The action mask (argmax positions, bounds, crew-collision, ship-cell
   lookup) is computed row-major. Ship-cell lookup uses a bit-pack: the 11
   grid rows of `ship` are packed to 11 integers per board row via a
   matmul with powers-of-two weights, then per-row bits are extracted with
   integer shifts.
 - All 9 actions are handled in "grid order" j = (dx+1)*3 + (dy+1) on
   device; the host permutes columns back to the reference action order.
"""

import os
import sys

for _p in ("/opt/trn_rl_repo", "/root/.axon_site/_ro/trn_rl_repo"):
    if os.path.isdir(_p) and _p not in sys.path:
        sys.path.insert(0, _p)

import numpy as np

import concourse.bass as bass
import concourse.mybir as mybir
from concourse import bacc
from concourse.alu_op_type import AluOpType as Op
from concourse.tile import TileContext
from concourse.bass_utils import run_bass_kernel_spmd

F32 = mybir.dt.float32
I32 = mybir.dt.int32
AX = mybir.AxisListType
AF = mybir.ActivationFunctionType

B = 262144
N_CORES = 8
GRID = 11
NFEAT = 121

# Actions in reference order, and the grid-order permutation.
ACTIONS = [(0, 0), (0, 1), (0, -1), (1, 0), (-1, 0), (1, 1), (1, -1), (-1, 1), (-1, -1)]
# grid order index j = (dx+1)*3 + (dy+1)
GRID_OF_ACTION = [(dx + 1) * 3 + (dy + 1) for dx, dy in ACTIONS]
DX_G = [d - 1 for d in range(3) for _ in range(3)]  # per grid j
DY_G = [e - 1 for _ in range(3) for e in range(3)]

G = 4  # rows per partition per group
GROUP_ROWS = 128 * G
FD = G * NFEAT  # free dim of a row-major group tile

# Matmul-path dtype: bf16 streams 1 col/cycle on the PE vs 4 for fp32.
USE_BF16 = os.environ.get("BOT_MM_DTYPE", "bf16") == "bf16"
BF16 = mybir.dt.bfloat16
MMDT = BF16 if USE_BF16 else F32


def _consts_f32():
    """Constant SBUF blobs, replicated across partitions where per-row."""
    c = {}
    c["ident"] = np.eye(128, dtype=np.float32)
    c["iota121"] = np.tile(np.arange(NFEAT, dtype=np.float32), (128, 1))
    c["iota11"] = np.tile(np.arange(GRID, dtype=np.float32), (128, 1))
    dx = np.array(DX_G, dtype=np.float32)
    dy = np.array(DY_G, dtype=np.float32)
    c["dxg"] = np.tile(dx, (128, 1))
    c["dyg"] = np.tile(dy, (128, 1))
    c["lox"] = np.tile(np.maximum(0.0, -dx), (128, 1))
    c["hix"] = np.tile(np.minimum(10.0, 10.0 - dx), (128, 1))
    c["loy"] = np.tile(np.maximum(0.0, -dy), (128, 1))
    c["hiy"] = np.tile(np.minimum(10.0, 10.0 - dy), (128, 1))
    return c


def _phi():
    """[121, 11] pack matrix: phi[11*k + y, k] = 2**y."""
    phi = np.zeros((NFEAT, GRID), dtype=np.float32)
    for k in range(GRID):
        for y in range(GRID):
            phi[GRID * k + y, k] = float(2 ** y)
    return phi


def build_nc(rows_per_core):
    assert rows_per_core % GROUP_ROWS == 0
    ng = rows_per_core // GROUP_ROWS

    nc = bacc.Bacc("TRN2", target_bir_lowering=False, debug=False,
                   num_devices=N_CORES)

    def din(name, shape, dt=F32):
        return nc.dram_tensor(name, shape, dt, kind="ExternalInput").ap()

    bot_d = din("bot", [rows_per_core, NFEAT])
    crew_d = din("crew", [rows_per_core, NFEAT])
    ship_d = din("ship", [rows_per_core, NFEAT])
    out_d = nc.dram_tensor("out", [rows_per_core, 9], F32,
                           kind="ExternalOutput").ap()

    wbot_d = din("wbot", [NFEAT, 64], MMDT)
    wcrew_d = din("wcrew", [NFEAT, 64], MMDT)
    wship_d = din("wship", [NFEAT, 64], MMDT)
    phi_d = din("phi", [NFEAT, GRID], MMDT)
    fc1a_d = din("fc1a", [128, 128], MMDT)
    fc1b_d = din("fc1b", [64, 128], MMDT)
    fc2w_d = din("fc2w", [128, 64], MMDT)
    outwb_d = din("outwb", [65, 9], MMDT)
    bias_bc_d = din("bias_bc", [128, 1])
    bias_s_d = din("bias_s", [64, 1])
    bias_f1_d = din("bias_f1", [128, 1])
    bias_f2_d = din("bias_f2", [64, 1])
    ident_d = din("ident", [128, 128])
    iota121_d = din("iota121", [128, NFEAT], MMDT)
    iota11_d = din("iota11", [128, GRID])
    dxg_d = din("dxg", [128, 9])
    dyg_d = din("dyg", [128, 9])
    lox_d = din("lox", [128, 9])
    hix_d = din("hix", [128, 9])
    loy_d = din("loy", [128, 9])
    hiy_d = din("hiy", [128, 9])
    dysi_d = din("dysi", [128, 3], I32)

    # Row-block views: [ng, 128, G*121]; partition p holds rows p*G..p*G+G-1.
    bot_v = bot_d.rearrange("(n p g) d -> n p (g d)", p=128, g=G)
    crew_v = crew_d.rearrange("(n p g) d -> n p (g d)", p=128, g=G)
    ship_v = ship_d.rearrange("(n p g) d -> n p (g d)", p=128, g=G)
    out_v = out_d.rearrange("(n p g) d -> n p (g d)", p=128, g=G)

    NB = 128 * G  # feature-major batch width per group

    with TileContext(nc) as tc:
        with tc.tile_pool(name="const", bufs=1) as cp, \
             tc.tile_pool(name="io", bufs=2) as iop, \
             tc.tile_pool(name="xt", bufs=2) as xtp, \
             tc.tile_pool(name="act", bufs=2) as actp, \
             tc.tile_pool(name="mask", bufs=2) as mkp, \
             tc.tile_pool(name="psxt", bufs=1, space="PSUM") as psxt, \
             tc.tile_pool(name="psmm", bufs=2, space="PSUM") as psmm, \
             tc.tile_pool(name="pssm", bufs=2, space="PSUM") as pssm:

            _ctr = [0]

            def ctile(dram, shape, dt=F32):
                _ctr[0] += 1
                t = cp.tile(shape, dt, tag=f"const{_ctr[0]}")
                nc.sync.dma_start(out=t[:], in_=dram)
                return t

            wbot = ctile(wbot_d, [NFEAT, 64], MMDT)
            wcrew = ctile(wcrew_d, [NFEAT, 64], MMDT)
            wship = ctile(wship_d, [NFEAT, 64], MMDT)
            phi = ctile(phi_d, [NFEAT, GRID], MMDT)
            fc1a = ctile(fc1a_d, [128, 128], MMDT)
            fc1b = ctile(fc1b_d, [64, 128], MMDT)
            fc2w = ctile(fc2w_d, [128, 64], MMDT)
            outwb = ctile(outwb_d, [65, 9], MMDT)
            bias_bc = ctile(bias_bc_d, [128, 1])
            bias_s = ctile(bias_s_d, [64, 1])
            bias_f1 = ctile(bias_f1_d, [128, 1])
            bias_f2 = ctile(bias_f2_d, [64, 1])
            ident = ctile(ident_d, [128, 128])
            iota121 = ctile(iota121_d, [128, NFEAT], MMDT)
            iota11 = ctile(iota11_d, [128, GRID])
            dxg = ctile(dxg_d, [128, 9])
            dyg = ctile(dyg_d, [128, 9])
            lox = ctile(lox_d, [128, 9])
            hix = ctile(hix_d, [128, 9])
            loy = ctile(loy_d, [128, 9])
            hiy = ctile(hiy_d, [128, 9])
            dysi = ctile(dysi_d, [128, 3], I32)

            def bc_g(ap2d, n):
                # [128, G] -> [128, G, n] stride-0 broadcast
                return ap2d.unsqueeze(2).broadcast_to([128, G, n])

            def bc_c(ap2d, n=None):
                # const [128, K] -> [128, G, K]
                k = ap2d.shape[1]
                return ap2d.unsqueeze(1).broadcast_to([128, G, k])

            for gi in range(ng):
                # ---- loads (row-major fp32) ----
                bot_rm = iop.tile([128, FD], F32, tag="bot")
                crew_rm = iop.tile([128, FD], F32, tag="crew")
                ship_rm = iop.tile([128, FD], F32, tag="ship")
                nc.sync.dma_start(out=bot_rm[:], in_=bot_v[gi])
                nc.sync.dma_start(out=crew_rm[:], in_=crew_v[gi])
                nc.sync.dma_start(out=ship_rm[:], in_=ship_v[gi])

                # ---- transposes to feature-major ----
                def transpose_in(rm, tag):
                    ps = psxt.tile([128, NB], F32, tag="psxt_" + tag)
                    for j in range(G):
                        nc.tensor.transpose(
                            out=ps[0:NFEAT, j * 128:(j + 1) * 128],
                            in_=rm[:, j * NFEAT:(j + 1) * NFEAT],
                            identity=ident[:],
                        )
                    sb = xtp.tile([128, NB], MMDT, tag="xt_" + tag)
                    nc.scalar.activation(out=sb[0:NFEAT, :], in_=ps[0:NFEAT, :],
                                         func=AF.Copy)
                    return sb

                xtb = transpose_in(bot_rm, "b")
                xtc = transpose_in(crew_rm, "c")
                xts = transpose_in(ship_rm, "s")

                # ---- encoder layer (feature-major) ----
                enc_ps = psmm.tile([128, NB], F32, tag="mm")
                nc.tensor.matmul(out=enc_ps[0:64, :], lhsT=wbot[:],
                                 rhs=xtb[0:NFEAT, :], start=True, stop=True)
                nc.tensor.matmul(out=enc_ps[64:128, :], lhsT=wcrew[:],
                                 rhs=xtc[0:NFEAT, :], start=True, stop=True)
                encs_ps = psmm.tile([128, NB], F32, tag="mm")
                nc.tensor.matmul(out=encs_ps[0:64, :], lhsT=wship[:],
                                 rhs=xts[0:NFEAT, :], start=True, stop=True)

                # ship bit-packs, row-major: P[r, k] = sum_y ship[r,11k+y]*2^y
                pp_ps = pssm.tile([128, 512], F32, tag="sm")
                for j in range(G):
                    nc.tensor.matmul(out=pp_ps[:, j * 128:j * 128 + GRID],
                                     lhsT=xts[0:NFEAT, j * 128:(j + 1) * 128],
                                     rhs=phi[:], start=True, stop=True)

                enc_a = actp.tile([128, NB], MMDT, tag="enc_a")
                nc.scalar.activation(out=enc_a[:], in_=enc_ps[:],
                                     func=AF.Relu, bias=bias_bc[:, 0:1])
                enc_b = actp.tile([128, NB], MMDT, tag="enc_b")
                nc.scalar.activation(out=enc_b[0:64, :], in_=encs_ps[0:64, :],
                                     func=AF.Relu, bias=bias_s[0:64, 0:1])

                # ---- fc1 / fc2 ----
                fc1_ps = psmm.tile([128, NB], F32, tag="mm")
                nc.tensor.matmul(out=fc1_ps[:], lhsT=fc1a[:], rhs=enc_a[:],
                                 start=True, stop=False)
                nc.tensor.matmul(out=fc1_ps[:], lhsT=fc1b[0:64, :],
                                 rhs=enc_b[0:64, :], start=False, stop=True)
                x1 = actp.tile([128, NB], MMDT, tag="x1")
                nc.scalar.activation(out=x1[:], in_=fc1_ps[:], func=AF.Relu,
                                     bias=bias_f1[:, 0:1])

                fc2_ps = psmm.tile([128, NB], F32, tag="mm")
                nc.tensor.matmul(out=fc2_ps[0:64, :], lhsT=fc2w[:], rhs=x1[:],
                                 start=True, stop=True)
                x2p = actp.tile([128, NB], MMDT, tag="x2p")
                nc.scalar.activation(out=x2p[0:64, :], in_=fc2_ps[0:64, :],
                                     func=AF.Relu, bias=bias_f2[0:64, 0:1])
                nc.gpsimd.memset(x2p[64:65, :], 1.0)

                # ---- output layer, flipped: logits land row-major ----
                log_ps = pssm.tile([128, 512], F32, tag="sm")
                for j in range(G):
                    nc.tensor.matmul(out=log_ps[:, j * 128:j * 128 + 9],
                                     lhsT=x2p[0:65, j * 128:(j + 1) * 128],
                                     rhs=outwb[0:65, :], start=True, stop=True)

                # ---- mask: argmax positions ----
                def argpos(rm, tag):
                    rmx = mkp.tile([128, G], F32, tag="rmx" + tag)
                    nc.vector.tensor_reduce(
                        out=rmx[:], in_=rm[:].rearrange("p (g d) -> p g d", g=G),
                        axis=AX.X, op=Op.max)
                    eq = mkp.tile([128, FD], MMDT, tag="eq" + tag)
                    nc.vector.tensor_tensor(
                        out=eq[:].rearrange("p (g d) -> p g d", g=G),
                        in0=rm[:].rearrange("p (g d) -> p g d", g=G),
                        in1=bc_g(rmx[:], NFEAT), op=Op.is_equal)
                    pm = mkp.tile([128, FD], MMDT, tag="pm" + tag)
                    nc.vector.scalar_tensor_tensor(
                        out=pm[:].rearrange("p (g d) -> p g d", g=G),
                        in0=eq[:].rearrange("p (g d) -> p g d", g=G),
                        scalar=-128.0, op0=Op.mult,
                        in1=bc_c(iota121[:]), op1=Op.add)
                    posr = mkp.tile([128, G], MMDT, tag="posr" + tag)
                    nc.vector.tensor_reduce(
                        out=posr[:], in_=pm[:].rearrange("p (g d) -> p g d", g=G),
                        axis=AX.X, op=Op.min)
                    # pos in [0,120]; x = pos // 11, y = pos mod 11 without a
                    # mod op: divide with +0.045 margin, cast (trunc OR
                    # round-to-nearest both land on x or x+1), then correct
                    # overshoot by checking the remainder sign.
                    posf = mkp.tile([128, G], F32, tag="posf" + tag)
                    nc.vector.tensor_scalar(out=posf[:], in0=posr[:],
                                            scalar1=128.0, scalar2=None,
                                            op0=Op.add)
                    xq = mkp.tile([128, G], F32, tag="xq" + tag)
                    nc.vector.tensor_scalar(out=xq[:], in0=posf[:],
                                            scalar1=1.0 / 11.0, scalar2=0.045,
                                            op0=Op.mult, op1=Op.add)
                    c1 = mkp.tile([128, G], I32, tag="c1" + tag)
                    nc.vector.tensor_copy(out=c1[:], in_=xq[:])
                    c1f = mkp.tile([128, G], F32, tag="c1f" + tag)
                    nc.vector.tensor_copy(out=c1f[:], in_=c1[:])
                    r1 = mkp.tile([128, G], F32, tag="r1" + tag)
                    nc.vector.scalar_tensor_tensor(
                        out=r1[:], in0=c1f[:], scalar=-11.0, op0=Op.mult,
                        in1=posf[:], op1=Op.add)
                    neg = mkp.tile([128, G], F32, tag="neg" + tag)
                    nc.vector.tensor_scalar(out=neg[:], in0=r1[:],
                                            scalar1=-0.5, scalar2=None,
                                            op0=Op.is_lt)
                    xf = mkp.tile([128, G], F32, tag="xf" + tag)
                    nc.vector.tensor_tensor(out=xf[:], in0=c1f[:],
                                            in1=neg[:], op=Op.subtract)
                    yf = mkp.tile([128, G], F32, tag="yf" + tag)
                    nc.vector.scalar_tensor_tensor(
                        out=yf[:], in0=xf[:], scalar=-11.0, op0=Op.mult,
                        in1=posf[:], op1=Op.add)
                    return xf, yf

                bxf, byf = argpos(bot_rm, "b")
                cxf, cyf = argpos(crew_rm, "c")
                byi = mkp.tile([128, G], I32, tag="byi")
                nc.vector.tensor_copy(out=byi[:], in_=byf[:])

                # ---- not-crew: (cx-bx, cy-by) != (dx, dy) ----
                dxv = mkp.tile([128, G], F32, tag="dxv")
                nc.vector.tensor_tensor(out=dxv[:], in0=cxf[:], in1=bxf[:],
                                        op=Op.subtract)
                dyv = mkp.tile([128, G], F32, tag="dyv")
                nc.vector.tensor_tensor(out=dyv[:], in0=cyf[:], in1=byf[:],
                                        op=Op.subtract)
                NA = G * 9
                d1 = mkp.tile([128, NA], F32, tag="d1")
                v3 = lambda t: t[:].rearrange("p (g a) -> p g a", a=9)
                nc.vector.tensor_tensor(out=v3(d1), in0=bc_g(dxv[:], 9),
                                        in1=bc_c(dxg[:]), op=Op.subtract)
                s1 = mkp.tile([128, NA], F32, tag="s1")
                nc.vector.tensor_tensor(out=s1[:], in0=d1[:], in1=d1[:],
                                        op=Op.mult)
                d2 = mkp.tile([128, NA], F32, tag="d2")
                nc.vector.tensor_tensor(out=v3(d2), in0=bc_g(dyv[:], 9),
                                        in1=bc_c(dyg[:]), op=Op.subtract)
                s2 = mkp.tile([128, NA], F32, tag="s2")
                nc.vector.tensor_tensor(out=s2[:], in0=d2[:], in1=d2[:],
                                        op=Op.mult)
                ss = mkp.tile([128, NA], F32, tag="ss")
                nc.vector.tensor_tensor(out=ss[:], in0=s1[:], in1=s2[:],
                                        op=Op.add)
                ncv = mkp.tile([128, NA], F32, tag="ncv")
                nc.vector.tensor_scalar(out=ncv[:], in0=ss[:], scalar1=0.5,
                                        scalar2=None, op0=Op.is_ge)

                # ---- in-bounds: lox <= bx <= hix and loy <= by <= hiy ----
                e1 = mkp.tile([128, NA], F32, tag="e1")
                nc.vector.tensor_tensor(out=v3(e1), in0=bc_g(bxf[:], 9),
                                        in1=bc_c(lox[:]), op=Op.subtract)
                e2 = mkp.tile([128, NA], F32, tag="e2")
                nc.vector.tensor_tensor(out=v3(e2), in0=bc_c(hix[:]),
                                        in1=bc_g(bxf[:], 9), op=Op.subtract)
                p1 = mkp.tile([128, NA], F32, tag="p1")
                nc.vector.tensor_tensor(out=p1[:], in0=e1[:], in1=e2[:],
                                        op=Op.mult)
                ibx = mkp.tile([128, NA], F32, tag="ibx")
                nc.vector.tensor_scalar(out=ibx[:], in0=p1[:], scalar1=-0.25,
                                        scalar2=None, op0=Op.is_ge)
                e3 = mkp.tile([128, NA], F32, tag="e3")
                nc.vector.tensor_tensor(out=v3(e3), in0=bc_g(byf[:], 9),
                                        in1=bc_c(loy[:]), op=Op.subtract)
                e4 = mkp.tile([128, NA], F32, tag="e4")
                nc.vector.tensor_tensor(out=v3(e4), in0=bc_c(hiy[:]),
                                        in1=bc_g(byf[:], 9), op=Op.subtract)
                p2 = mkp.tile([128, NA], F32, tag="p2")
                nc.vector.tensor_tensor(out=p2[:], in0=e3[:], in1=e4[:],
                                        op=Op.mult)
                iby = mkp.tile([128, NA], F32, tag="iby")
                nc.vector.tensor_scalar(out=iby[:], in0=p2[:], scalar1=-0.25,
                                        scalar2=None, op0=Op.is_ge)
                ibv = mkp.tile([128, NA], F32, tag="ibv")
                nc.vector.tensor_tensor(out=ibv[:], in0=ibx[:], in1=iby[:],
                                        op=Op.mult)

                # ---- ship bit at target cell ----
                ppad = mkp.tile([128, G * 13], F32, tag="ppad")
                nc.gpsimd.memset(ppad[:], 0.0)
                pp_v = bass.AP(pp_ps[:].tensor, pp_ps[:].offset,
                               [pp_ps[:].ap[0], [128, G], [1, GRID]])
                nc.vector.tensor_copy(
                    out=ppad[:].rearrange("p (g w) -> p g w", w=13)[:, :, 1:12],
                    in_=pp_v)
                ohx = mkp.tile([128, G * GRID], F32, tag="ohx")
                nc.vector.tensor_tensor(
                    out=ohx[:].rearrange("p (g w) -> p g w", w=GRID),
                    in0=bc_c(iota11[:]), in1=bc_g(bxf[:], GRID), op=Op.is_equal)
                # window product: prod[g, d, y] = ohx[g, y] * ppad[g, d + y]
                prod = mkp.tile([128, G * 33], F32, tag="prodw")
                ohx_b = bass.AP(ohx[:].tensor, ohx[:].offset,
                                [ohx[:].ap[0], [GRID, G], [0, 3], [1, GRID]])
                ppad_w = bass.AP(ppad[:].tensor, ppad[:].offset,
                                 [ppad[:].ap[0], [13, G], [1, 3], [1, GRID]])
                nc.vector.tensor_tensor(
                    out=prod[:].rearrange("p (g d y) -> p g d y", d=3, y=GRID),
                    in0=ohx_b, in1=ppad_w, op=Op.mult)
                pd = mkp.tile([128, G * 3], F32, tag="pd")
                nc.vector.tensor_reduce(
                    out=pd[:], in_=prod[:].rearrange("p (g d y) -> p g d y",
                                                     d=3, y=GRID),
                    axis=AX.X, op=Op.add)
                pdi = mkp.tile([128, G * 3], I32, tag="pdi")
                nc.vector.tensor_copy(out=pdi[:], in_=pd[:])
                sh = mkp.tile([128, G * 3], I32, tag="sh")
                nc.vector.tensor_tensor(
                    out=sh[:].rearrange("p (g e) -> p g e", e=3),
                    in0=byi[:].unsqueeze(2).broadcast_to([128, G, 3]),
                    in1=dysi[:].unsqueeze(1).broadcast_to([128, G, 3]),
                    op=Op.add)
                shc = mkp.tile([128, G * 3], I32, tag="shc")
                nc.vector.tensor_scalar(out=shc[:], in0=sh[:], scalar1=0,
                                        scalar2=None, op0=Op.max)
                qb = mkp.tile([128, NA], I32, tag="qb")
                pdi3 = pdi[:].rearrange("p (g d) -> p g d", d=3)
                shc3 = shc[:].rearrange("p (g e) -> p g e", e=3)
                nc.vector.tensor_tensor(
                    out=qb[:].rearrange("p (g d e) -> p g d e", d=3, e=3),
                    in0=pdi3.unsqueeze(3).broadcast_to([128, G, 3, 3]),
                    in1=bass.AP(shc3.tensor, shc3.offset,
                                [shc3.ap[0], shc3.ap[1], [0, 3], shc3.ap[2]]),
                    op=Op.logical_shift_right)
                svi = mkp.tile([128, NA], I32, tag="svi")
                nc.vector.tensor_scalar(out=svi[:], in0=qb[:], scalar1=1,
                                        scalar2=None, op0=Op.bitwise_and)

                # ---- combine mask (int32) and apply ----
                v1 = mkp.tile([128, NA], F32, tag="v1")
                nc.vector.tensor_tensor(out=v1[:], in0=ibv[:], in1=ncv[:],
                                        op=Op.mult)
                v1i = mkp.tile([128, NA], I32, tag="v1i")
                nc.vector.tensor_copy(out=v1i[:], in_=v1[:])
                v2 = mkp.tile([128, NA], I32, tag="v2")
                nc.vector.tensor_tensor(out=v2[:], in0=v1i[:], in1=svi[:],
                                        op=Op.mult)
                fin = mkp.tile([128, NA], F32, tag="fin")
                nc.gpsimd.memset(fin[:], float("-inf"))
                log_v = bass.AP(log_ps[:].tensor, log_ps[:].offset,
                                [log_ps[:].ap[0], [128, G], [1, 9]])
                nc.vector.copy_predicated(
                    out=fin[:].rearrange("p (g a) -> p g a", a=9),
                    mask=v2[:].rearrange("p (g a) -> p g a", a=9),
                    data=log_v)
                nc.sync.dma_start(out=out_v[gi], in_=fin[:])

    nc.compile()
    return nc


_NC_CACHE = {}


def _get_nc(rows_per_core):
    if rows_per_core not in _NC_CACHE:
        _NC_CACHE[rows_per_core] = build_nc(rows_per_core)
    return _NC_CACHE[rows_per_core]


def _in_maps(inputs, rows_per_core):
    bot = np.ascontiguousarray(inputs["bot"], dtype=np.float32)
    crew = np.ascontiguousarray(inputs["crew"], dtype=np.float32)
    ship = np.ascontiguousarray(inputs["ship"], dtype=np.float32)

    out_w = np.asarray(inputs["out_w"], dtype=np.float32)
    out_b = np.asarray(inputs["out_b"], dtype=np.float32)
    # permute output layer to grid order
    perm = np.array(GRID_OF_ACTION)  # grid index per action
    inv = np.argsort(perm)  # action index per grid slot
    out_w_g = out_w[:, inv]
    out_b_g = out_b[inv]

    fc1_w = np.asarray(inputs["fc1_w"], dtype=np.float32)
    shared = {
        "wbot": np.asarray(inputs["bot_w"], dtype=np.float32),
        "wcrew": np.asarray(inputs["crew_w"], dtype=np.float32),
        "wship": np.asarray(inputs["ship_w"], dtype=np.float32),
        "phi": _phi(),
        "fc1a": np.ascontiguousarray(fc1_w[0:128, :]),
        "fc1b": np.ascontiguousarray(fc1_w[128:192, :]),
        "fc2w": np.asarray(inputs["fc2_w"], dtype=np.float32),
        "outwb": np.vstack([out_w_g, out_b_g[None, :]]).astype(np.float32),
        "bias_bc": np.concatenate([inputs["bot_b"], inputs["crew_b"]])
                     .astype(np.float32)[:, None],
        "bias_s": np.asarray(inputs["ship_b"], dtype=np.float32)[:, None],
        "bias_f1": np.asarray(inputs["fc1_b"], dtype=np.float32)[:, None],
        "bias_f2": np.asarray(inputs["fc2_b"], dtype=np.float32)[:, None],
        "dysi": np.tile(np.array([-1, 0, 1], dtype=np.int32), (128, 1)),
    }
    for k, v in _consts_f32().items():
        shared[k] = v

    if USE_BF16:
        import ml_dtypes
        for k in ("wbot", "wcrew", "wship", "phi", "fc1a", "fc1b", "fc2w",
                  "outwb", "iota121"):
            shared[k] = shared[k].astype(ml_dtypes.bfloat16)

    maps = []
    for c in range(N_CORES):
        lo, hi = c * rows_per_core, (c + 1) * rows_per_core
        m = dict(shared)
        m["bot"] = bot[lo:hi]
        m["crew"] = crew[lo:hi]
        m["ship"] = ship[lo:hi]
        maps.append(m)
    return maps


def run(inputs, rows_per_core, trace=False):
    nc = _get_nc(rows_per_core)
    maps = _in_maps(inputs, rows_per_core)
    res = run_bass_kernel_spmd(nc, maps, list(range(N_CORES)), trace=trace)
    out_g = np.concatenate([res.results[c]["out"] for c in range(N_CORES)],
                           axis=0)
    # grid order -> action order
    out = out_g[:, np.array(GRID_OF_ACTION)]
    return out, res


def kernel(**inputs):
    rows = np.asarray(inputs["bot"]).shape[0]
    out, _ = run(inputs, rows // N_CORES, trace=False)
    return out.astype(np.float32)
